# revision 1
# baseline (speedup 1.0000x reference)
# Multi-head attention (B=2, S=2048, D=1024, H=16) on 8 TRN2 NeuronCores.
#
# Sharding (hardcoded): core c in [0..8) handles batch b = c//4 and head
# group g = c%4 (4 heads = 256 output features of wq/wk/wv, 256 input rows
# of wo). Each core computes a partial output projection [S, D]; the host
# sums the 4 partials per batch and adds wo_bias (row-parallel unshard).
#
# Device-side layout choices:
#   - activations enter transposed ([D, S]) so every matmul contracts over
#     the partition axis with no on-device transposes;
#   - scores are computed transposed (S^T[k, q]) so softmax(P) feeds the
#     P@V matmul directly (contraction over k on partitions);
#   - the softmax denominator comes free as an extra ones-column appended
#     to each head's V block (output row 64 of the PV accumulation);
#   - matmuls run in float32r (full-rate fp32 path for moving dim >= 256);
#     P/V/out-proj run in bf16.
import functools
import sys

import numpy as np

try:
    import concourse  # noqa: F401
except ImportError:  # harness env without the default path
    sys.path.insert(0, "/opt/trn_rl_repo")
    sys.path.insert(0, "/opt/pypackages")

import ml_dtypes

BF16 = ml_dtypes.bfloat16

B, S, D, H = 2, 2048, 1024, 16
HD = D // H          # 64
NCORES = 8
GH = 4               # head groups (tensor-parallel)
HPG = H // GH        # heads per group = 4
DG = D // GH         # features per group = 256
P = 128              # partitions
TDIN = D // P        # 8 din tiles
SC = 4               # s-chunks of 512 for projections
CW = S // SC         # 512
QC = 2               # q-chunks of 1024 for attention
QW = S // QC         # 1024
KT = S // P          # 16 k tiles
NT2 = DG // P        # 2 dout tiles per group


def build_graph():
    """Build the SPMD Bass graph (identical on all 8 cores)."""
    from contextlib import ExitStack

    from concourse import bacc, mybir, tile

    f32 = mybir.dt.float32
    f32r = mybir.dt.float32r
    bf16 = mybir.dt.bfloat16
    EXP = mybir.ActivationFunctionType.Exp

    nc = bacc.Bacc(
        "TRN2", target_bir_lowering=False, debug=False, num_devices=NCORES
    )

    xq = nc.dram_tensor("xq_t", (P, TDIN, S), bf16, kind="ExternalInput")
    xk = nc.dram_tensor("xk_t", (P, TDIN, S), bf16, kind="ExternalInput")
    xv = nc.dram_tensor("xv_t", (P, TDIN, S), bf16, kind="ExternalInput")
    mk = nc.dram_tensor("mask_t", (S, S), bf16, kind="ExternalInput")
    wq = nc.dram_tensor("wq", (P, TDIN, DG), bf16, kind="ExternalInput")
    wk = nc.dram_tensor("wk", (P, TDIN, DG), bf16, kind="ExternalInput")
    wv = nc.dram_tensor("wv", (P, TDIN, DG), bf16, kind="ExternalInput")
    # wo pre-arranged host-side to [64, HPG, D] (j, h, n) so each head's
    # 64 rows sit on partitions 0..63.
    wo = nc.dram_tensor("wo", (HD, HPG, D), bf16, kind="ExternalInput")
    qb = nc.dram_tensor("qb", (1, DG), bf16, kind="ExternalInput")
    kb = nc.dram_tensor("kb", (1, DG), bf16, kind="ExternalInput")
    vb = nc.dram_tensor("vb", (1, DG), bf16, kind="ExternalInput")
    out = nc.dram_tensor("out", (S, D), bf16, kind="ExternalOutput")

    with tile.TileContext(nc) as tc, ExitStack() as ctx:
        wpool = ctx.enter_context(tc.tile_pool(name="wpool", bufs=1))
        cpool = ctx.enter_context(tc.tile_pool(name="cpool", bufs=1))
        qkpool = ctx.enter_context(tc.tile_pool(name="qk", bufs=1))
        vpool = ctx.enter_context(tc.tile_pool(name="vsb", bufs=1))
        mpool = ctx.enter_context(tc.tile_pool(name="msk", bufs=1))
        ppool = ctx.enter_context(tc.tile_pool(name="ptile", bufs=3))
        spool = ctx.enter_context(tc.tile_pool(name="small", bufs=2))
        dpool = ctx.enter_context(tc.tile_pool(name="dscr", bufs=2, space="DRAM"))
        bigps = ctx.enter_context(tc.tile_pool(name="bigps", bufs=3, space="PSUM"))
        ops_pool = ctx.enter_context(tc.tile_pool(name="ops", bufs=1, space="PSUM"))

        # ---- persistent SBUF tensors -------------------------------------
        wq_sb = wpool.tile([P, TDIN, DG], bf16)
        wk_sb = wpool.tile([P, TDIN, DG], bf16)
        wv_sb = wpool.tile([P, TDIN, DG], bf16)
        for wsb_, wdr_ in ((wq_sb, wq), (wk_sb, wk), (wv_sb, wv)):
            for th_ in range(2):
                nc.sync.dma_start(
                    wsb_[:, th_ * 4 : (th_ + 1) * 4, :],
                    wdr_.ap()[:, th_ * 4 : (th_ + 1) * 4, :],
                )
        wo_sb = wpool.tile([HD, HPG, D], bf16)
        nc.sync.dma_start(wo_sb[:], wo.ap())
        qb_sb = cpool.tile([1, DG], bf16)
        kb_sb = cpool.tile([1, DG], bf16)
        vb_sb = cpool.tile([1, DG], bf16)
        nc.sync.dma_start(qb_sb[:], qb.ap())
        nc.sync.dma_start(kb_sb[:], kb.ap())
        nc.sync.dma_start(vb_sb[:], vb.ap())
        # ones: row 0 used as [1, CW] rhs / [1, P] lhsT at partition 0;
        # row 64 used as [1, HD] lhsT at partition 64 (denominator bcast).
        ones2 = cpool.tile([1, CW], bf16)
        nc.vector.memset(ones2[:], 1.0)

        qT_sb = qkpool.tile([P, NT2, S], bf16)   # q projection, transposed
        kT_sb = qkpool.tile([P, NT2, S], bf16)
        # v blocks: per k-tile, per head: [v(64) | ones] -> 65 cols
        v_sb = vpool.tile([P, KT, HPG * (HD + 1)], bf16)
        nc.vector.memset(
            v_sb[:].rearrange("p s (h x) -> p s h x", h=HPG)[:, :, :, HD : HD + 1],
            1.0,
        )
        # ---- projections -------------------------------------------------
        # q, k: out qT[dout, s] = wq^T(stationary) x q^T(moving) + bias
        xpool_cm = tc.tile_pool(name="xin", bufs=2)
        xpool = xpool_cm.__enter__()
        NCH = S // 1024
        for xdram, wsb, bias_sb, dest in (
            (xq, wq_sb, qb_sb, qT_sb),
            (xk, wk_sb, kb_sb, kT_sb),
        ):
            for sc in range(NCH):
                xch = xpool.tile([P, TDIN, 1024], bf16, tag="xch")
                for th_ in range(4):
                    nc.sync.dma_start(
                        xch[:, th_ * 2 : (th_ + 1) * 2, :],
                        xdram.ap()[
                            :, th_ * 2 : (th_ + 1) * 2, sc * 1024 : (sc + 1) * 1024
                        ],
                    )
                for half in range(2):
                    s0 = sc * 1024 + half * 512
                    for dt in range(NT2):
                        ps = bigps.tile(
                            [P, CW], f32, tag="ps", name=f"pj_{sc}_{half}_{dt}"
                        )
                        for ktl in range(TDIN):
                            nc.tensor.matmul(
                                ps[:],
                                lhsT=wsb[:, ktl, dt * P : (dt + 1) * P],
                                rhs=xch[:, ktl, half * 512 : (half + 1) * 512],
                                start=(ktl == 0),
                                stop=False,
                            )
                        nc.tensor.matmul(
                            ps[:],
                            lhsT=bias_sb[0:1, dt * P : (dt + 1) * P],
                            rhs=ones2[0:1, :],
                            start=False,
                            stop=True,
                        )
                        nc.vector.tensor_copy(
                            dest[:, dt, s0 : s0 + 512], ps[:]
                        )
        # v: natural layout [s, dout] + bias, drained per-head with ones col
        for sc in range(NCH):
            xch = xpool.tile([P, TDIN, 1024], bf16, tag="xch")
            for th_ in range(4):
                nc.sync.dma_start(
                    xch[:, th_ * 2 : (th_ + 1) * 2, :],
                    xv.ap()[
                        :, th_ * 2 : (th_ + 1) * 2, sc * 1024 : (sc + 1) * 1024
                    ],
                )
            for m in range(1024 // P):
                st = sc * (1024 // P) + m
                ps = bigps.tile([P, DG], f32, tag="ps", name=f"pv_{sc}_{m}")
                for ktl in range(TDIN):
                    nc.tensor.matmul(
                        ps[:],
                        lhsT=xch[:, ktl, m * P : (m + 1) * P],
                        rhs=wv_sb[:, ktl, :],
                        start=(ktl == 0),
                        stop=False,
                    )
                nc.tensor.matmul(
                    ps[:],
                    lhsT=ones2[0:1, 0:P],
                    rhs=vb_sb[:],
                    start=False,
                    stop=True,
                )
                nc.vector.tensor_copy(
                    v_sb[:, st, :].rearrange("p (h x) -> p h x", h=HPG)[
                        :, :, 0:HD
                    ],
                    ps[:].rearrange("p (h x) -> p h x", h=HPG),
                )
        xpool_cm.__exit__(None, None, None)

        # mask load issued after projection DMAs so it doesn't hog queues
        mask_sb = mpool.tile([P, KT, S], bf16)
        mk_r = mk.ap().rearrange("(t p) q -> p t q", p=P)
        for kt in range(KT):
            nc.sync.dma_start(mask_sb[:, kt, :], mk_r[:, kt, :])

        # ---- attention ---------------------------------------------------
        # One head at a time; score psum triple-buffered so the PE can run
        # up to 3 k-tiles ahead of the exp/mask/PV chain.
        opool_sb = ctx.enter_context(tc.tile_pool(name="otn", bufs=1))
        otn_sb = opool_sb.tile([HD, HPG, S], bf16)

        def emit_outproj(st):
            osb2 = ppool.tile([P, D], bf16, tag="outsb", name=f"outsb_{st}")
            for nch in range(2):
                op_ps = bigps.tile(
                    [P, 512], f32, tag="ps", name=f"ops2_{st}_{nch}"
                )
                for h_ in range(HPG):
                    nc.tensor.matmul(
                        op_ps[:],
                        lhsT=otn_sb[:, h_, st * P : (st + 1) * P],
                        rhs=wo_sb[:, h_, nch * 512 : (nch + 1) * 512],
                        start=(h_ == 0),
                        stop=(h_ == HPG - 1),
                    )
                nc.vector.tensor_copy(
                    osb2[:, nch * 512 : (nch + 1) * 512], op_ps[:]
                )
            nc.sync.dma_start(out.ap()[st * P : (st + 1) * P, :], osb2[:])

        pending_st = []
        for qc in range(QC):
            for h in range(HPG):
                t, po = h // 2, (h % 2) * HD
                o_ps = ops_pool.tile(
                    [HD + 1, QW], f32, tag="ops", name=f"ops_{qc}_{h}"
                )
                for kt in range(KT):
                    s_ps = bigps.tile(
                        [P, QW], f32, tag="ps", name=f"sps_{qc}_{h}_{kt}"
                    )
                    for hf in range(2):
                        nc.tensor.matmul(
                            s_ps[:, hf * 512 : (hf + 1) * 512],
                            lhsT=kT_sb[po : po + HD, t, kt * P : (kt + 1) * P],
                            rhs=qT_sb[
                                po : po + HD,
                                t,
                                qc * QW + hf * 512 : qc * QW + (hf + 1) * 512,
                            ],
                            start=True,
                            stop=True,
                        )
                    pt = ppool.tile(
                        [P, QW], bf16, tag="p", name=f"pt_{qc}_{h}_{kt}"
                    )
                    nc.scalar.activation(pt[:], s_ps[:], EXP, scale=0.125)
                    nc.vector.tensor_mul(
                        pt[:], pt[:], mask_sb[:, kt, qc * QW : (qc + 1) * QW]
                    )
                    for hf in range(2):
                        nc.tensor.matmul(
                            o_ps[:, hf * 512 : (hf + 1) * 512],
                            lhsT=v_sb[:, kt, h * 65 : (h + 1) * 65],
                            rhs=pt[:, hf * 512 : (hf + 1) * 512],
                            start=(kt == 0),
                            stop=(kt == KT - 1),
                        )
                # softmax normalization (no PE): approx-recip of the
                # denominator row, DRAM-bounce broadcast, one TT multiply.
                rec65 = spool.tile([HD + 1, QW], f32, tag="rec")
                nc.vector.reciprocal_approx_fast(out=rec65[:], in_=o_ps[:])
                osb = spool.tile([HD, QW], f32, tag="osb")
                nc.vector.tensor_copy(osb[:], o_ps[0:HD, :])
                scr = dpool.tile([1, QW], f32, tag="scr", name=f"scr_{qc}_{h}")
                nc.sync.dma_start(scr[:], rec65[HD : HD + 1, :])
                rb = spool.tile([HD, QW], f32, tag="rb")
                nc.sync.dma_start(rb[:], scr[:].to_broadcast((HD, QW)))
                nc.vector.tensor_mul(
                    otn_sb[:, h, qc * QW : (qc + 1) * QW], osb[:], rb[:]
                )

            pending_st.extend(range(qc * (QW // P), (qc + 1) * (QW // P)))

        for st in pending_st:
            emit_outproj(st)

    nc.compile()
    return nc


@functools.lru_cache(maxsize=1)
def _graph():
    return build_graph()


def make_in_maps(
    query, key, value, mask,
    wq_kernel, wq_bias, wk_kernel, wk_bias,
    wv_kernel, wv_bias, wo_kernel, wo_bias,
):
    q = np.asarray(query, np.float32)
    k = np.asarray(key, np.float32)
    v = np.asarray(value, np.float32)
    mask = np.asarray(mask)
    wqk = np.asarray(wq_kernel, np.float32)
    wkk = np.asarray(wk_kernel, np.float32)
    wvk = np.asarray(wv_kernel, np.float32)
    wok = np.asarray(wo_kernel, np.float32)

    def tile_x(a):  # [S, D] -> [P, TDIN, S] pre-tiled transpose
        return np.ascontiguousarray(
            a.T.reshape(TDIN, P, S).transpose(1, 0, 2)
        ).astype(BF16)

    xt = [[tile_x(x[b]) for x in (q, k, v)] for b in range(B)]
    mt = [
        np.ascontiguousarray(mask[b].T.astype(np.float32)).astype(BF16)
        for b in range(B)
    ]
    in_maps = []
    for c in range(NCORES):
        b, g = divmod(c, GH)
        cs = slice(g * DG, (g + 1) * DG)
        wo_arr = np.ascontiguousarray(
            wok[cs, :].reshape(HPG, HD, D).transpose(1, 0, 2)
        ).astype(BF16)
        in_maps.append(
            {
                "xq_t": xt[b][0],
                "xk_t": xt[b][1],
                "xv_t": xt[b][2],
                "mask_t": mt[b],
                "wq": np.ascontiguousarray(wqk[:, cs].reshape(TDIN, P, DG).transpose(1, 0, 2)).astype(BF16),
                "wk": np.ascontiguousarray(wkk[:, cs].reshape(TDIN, P, DG).transpose(1, 0, 2)).astype(BF16),
                "wv": np.ascontiguousarray(wvk[:, cs].reshape(TDIN, P, DG).transpose(1, 0, 2)).astype(BF16),
                "wo": wo_arr,
                "qb": np.asarray(wq_bias, np.float32)[cs].reshape(1, DG).astype(BF16),
                "kb": np.asarray(wk_bias, np.float32)[cs].reshape(1, DG).astype(BF16),
                "vb": np.asarray(wv_bias, np.float32)[cs].reshape(1, DG).astype(BF16),
            }
        )
    return in_maps


def combine_outputs(results, wo_bias):
    outs = np.stack([np.asarray(r["out"], np.float32) for r in results])
    full = outs.reshape(B, GH, S, D).sum(axis=1)
    return (full + np.asarray(wo_bias, np.float32)[None, None, :]).astype(
        np.float32
    )


def kernel(**inputs):
    from concourse import bass_utils

    nc = _graph()
    in_maps = make_in_maps(**inputs)
    res = bass_utils.run_bass_kernel_spmd(
        nc, in_maps, core_ids=list(range(NCORES))
    )
    return combine_outputs(res.results, inputs["wo_bias"])



# revision 13
# speedup vs baseline: 1.1783x; 1.1783x over previous
# Multi-head attention (B=2, S=2048, D=1024, H=16) on 8 TRN2 NeuronCores.
#
# Sharding (hardcoded): core c in [0..8) handles batch b = c//4 and head
# group g = c%4 (4 heads = 256 output features of wq/wk/wv, 256 input rows
# of wo). Each core computes a partial output projection [S, D]; the host
# sums the 4 partials per batch and adds wo_bias (row-parallel unshard).
#
# Device-side schedule (v2):
#   - projections stream in two 1024-col chunks; attention for head 0
#     starts as soon as chunk 0 of k/v is resident, so the scalar engine
#     (exp - the throughput floor at 1 elem/cycle/lane) starts ~15us in;
#   - per head, score matmuls run two k-tiles ahead of the P@V matmuls so
#     the PE never waits on the exp->mask chain (s_ps double-buffered);
#   - softmax denominator via a ones-column appended to each head's V;
#   - output projection matmuls are interleaved into the second q-chunk's
#     attention as PE filler; biases fold into PSUM evacuation on the DVE.
import functools
import sys

import numpy as np

try:
    import concourse  # noqa: F401
except ImportError:  # harness env without the default path
    sys.path.insert(0, "/opt/trn_rl_repo")
    sys.path.insert(0, "/opt/pypackages")

import ml_dtypes

BF16 = ml_dtypes.bfloat16

B, S, D, H = 2, 2048, 1024, 16
HD = D // H          # 64
NCORES = 8
GH = 4               # head groups (tensor-parallel)
HPG = H // GH        # heads per group = 4
DG = D // GH         # features per group = 256
P = 128              # partitions
TDIN = D // P        # 8 din tiles
NCH = 2              # x-chunks of 1024 for projections
CW = S // NCH        # 1024
QC = 2               # q-chunks of 1024 for attention
QW = S // QC         # 1024
KT = S // P          # 16 k tiles
NT2 = DG // P        # 2 dout tiles per group


def build_graph():
    """Build the SPMD Bass graph (identical on all 8 cores)."""
    from contextlib import ExitStack

    from concourse import bacc, mybir, tile

    f32 = mybir.dt.float32
    bf16 = mybir.dt.bfloat16
    EXP = mybir.ActivationFunctionType.Exp

    nc = bacc.Bacc(
        "TRN2", target_bir_lowering=False, debug=False, num_devices=NCORES
    )

    xq = nc.dram_tensor("xq_t", (P, TDIN, S), bf16, kind="ExternalInput")
    xk = nc.dram_tensor("xk_t", (P, TDIN, S), bf16, kind="ExternalInput")
    xv = nc.dram_tensor("xv_t", (P, TDIN, S), bf16, kind="ExternalInput")
    mk = nc.dram_tensor("mask_t", (S, S), bf16, kind="ExternalInput")
    wq = nc.dram_tensor("wq", (P, TDIN, DG), bf16, kind="ExternalInput")
    wk = nc.dram_tensor("wk", (P, TDIN, DG), bf16, kind="ExternalInput")
    wv = nc.dram_tensor("wv", (P, TDIN, DG), bf16, kind="ExternalInput")
    # wo pre-arranged host-side to [64, HPG, D] (j, h, n) so each head's
    # 64 rows sit on partitions 0..63.
    wo = nc.dram_tensor("wo", (HD, HPG, D), bf16, kind="ExternalInput")
    qb = nc.dram_tensor("qb", (1, DG), bf16, kind="ExternalInput")
    kb = nc.dram_tensor("kb", (1, DG), bf16, kind="ExternalInput")
    vb = nc.dram_tensor("vb", (1, DG), bf16, kind="ExternalInput")
    out = nc.dram_tensor("out", (S, D), bf16, kind="ExternalOutput")

    with tile.TileContext(nc) as tc, ExitStack() as ctx:
        wpool = ctx.enter_context(tc.tile_pool(name="wpool", bufs=1))
        qkpool = ctx.enter_context(tc.tile_pool(name="qk", bufs=1))
        vpool = ctx.enter_context(tc.tile_pool(name="vsb", bufs=1))
        mpool = ctx.enter_context(tc.tile_pool(name="msk", bufs=1))
        xpool = ctx.enter_context(tc.tile_pool(name="xin", bufs=1))
        ptpool = ctx.enter_context(tc.tile_pool(name="ptile", bufs=3))
        npool = ctx.enter_context(tc.tile_pool(name="norm", bufs=1))
        otnpool = ctx.enter_context(tc.tile_pool(name="otn", bufs=1))
        outpool = ctx.enter_context(tc.tile_pool(name="outsb", bufs=2))
        dpool = ctx.enter_context(tc.tile_pool(name="dscr", bufs=2, space="DRAM"))
        # PSUM: 2x2 banks score double-buffer + 2 banks PV accum + 2 banks
        # scratch (projections early / out-proj late) = 8 banks exactly.
        sps_pool = ctx.enter_context(tc.tile_pool(name="sps", bufs=2, space="PSUM"))
        ops_pool = ctx.enter_context(tc.tile_pool(name="ops", bufs=1, space="PSUM"))
        scr_pool = ctx.enter_context(tc.tile_pool(name="scrps", bufs=2, space="PSUM"))

        # ---- persistent SBUF tensors -------------------------------------
        wq_sb = wpool.tile([P, TDIN, DG], bf16)
        wk_sb = wpool.tile([P, TDIN, DG], bf16)
        wv_sb = wpool.tile([P, TDIN, DG], bf16)
        for wsb_, wdr_ in ((wq_sb, wq), (wk_sb, wk), (wv_sb, wv)):
            for th_ in range(2):
                nc.sync.dma_start(
                    wsb_[:, th_ * 4 : (th_ + 1) * 4, :],
                    wdr_.ap()[:, th_ * 4 : (th_ + 1) * 4, :],
                )
        wo_sb = wpool.tile([HD, HPG, D], bf16)
        nc.sync.dma_start(wo_sb[:], wo.ap())
        qb_sb = wpool.tile([1, DG], bf16)
        kb_sb = wpool.tile([1, DG], bf16)
        vb_sb = wpool.tile([1, DG], bf16)
        nc.sync.dma_start(qb_sb[:], qb.ap())
        nc.sync.dma_start(kb_sb[:], kb.ap())
        nc.sync.dma_start(vb_sb[:], vb.ap())
        ones2 = wpool.tile([1, 512], bf16)
        nc.vector.memset(ones2[:], 1.0)

        qT_sb = qkpool.tile([P, NT2, S], bf16)   # q projection, transposed
        kT_sb = qkpool.tile([P, NT2, S], bf16)
        # v blocks: per k-tile, per head: [v(64) | ones] -> 65 cols
        v_sb = vpool.tile([P, KT, HPG * (HD + 1)], bf16)
        nc.vector.memset(
            v_sb[:].rearrange("p s (h x) -> p s h x", h=HPG)[:, :, :, HD : HD + 1],
            1.0,
        )
        mask_sb = mpool.tile([P, KT, S], bf16)
        mk_r = mk.ap().rearrange("(t p) q -> p t q", p=P)

        # warm the exp table set while DMAs stream (off critical path)
        warm = npool.tile([1, 32], bf16, tag="warm")
        nc.vector.memset(warm[:], 0.0)
        nc.scalar.activation(warm[:], warm[:], EXP)

        otn_sb = otnpool.tile([HD, HPG, S], bf16)

        # ---- emit helpers ------------------------------------------------
        def emit_x_dma(c):
            tiles = []
            for xdram, tag in ((xq, "xq"), (xk, "xk"), (xv, "xv")):
                t_ = xpool.tile([P, TDIN, CW], bf16, tag=tag)
                for th_ in range(4):
                    nc.sync.dma_start(
                        t_[:, th_ * 2 : (th_ + 1) * 2, :],
                        xdram.ap()[
                            :, th_ * 2 : (th_ + 1) * 2, c * CW : (c + 1) * CW
                        ],
                    )
                tiles.append(t_)
            return tiles

        def emit_mask_dma(kts, qc):
            for kt in kts:
                nc.sync.dma_start(
                    mask_sb[:, kt, qc * QW : (qc + 1) * QW],
                    mk_r[:, kt, qc * QW : (qc + 1) * QW],
                )

        def emit_proj_chunk(c, xq_c, xk_c, xv_c):
            # q,k: out qT[dout, s] = W^T(stationary) x x(moving); bias on evac
            for xch, wsb, bias_sb, dest in (
                (xq_c, wq_sb, qb_sb, qT_sb),
                (xk_c, wk_sb, kb_sb, kT_sb),
            ):
                for half in range(2):
                    s0 = c * CW + half * 512
                    for dt in range(NT2):
                        ps = scr_pool.tile(
                            [P, 512], f32, tag="ps", name=f"pj_{c}_{half}_{dt}"
                        )
                        for ktl in range(TDIN):
                            nc.tensor.matmul(
                                ps[:],
                                lhsT=wsb[:, ktl, dt * P : (dt + 1) * P],
                                rhs=xch[:, ktl, half * 512 : (half + 1) * 512],
                                start=(ktl == 0),
                                stop=False,
                            )
                        nc.tensor.matmul(
                            ps[:],
                            lhsT=bias_sb[0:1, dt * P : (dt + 1) * P],
                            rhs=ones2[0:1, :],
                            start=False,
                            stop=True,
                        )
                        nc.vector.tensor_copy(dest[:, dt, s0 : s0 + 512], ps[:])
            # v: natural layout [s_tile, dout]; bias via broadcast add on evac
            for m in range(CW // P):
                st = c * (CW // P) + m
                ps = scr_pool.tile([P, 512], f32, tag="ps", name=f"pv_{c}_{m}")
                for ktl in range(TDIN):
                    nc.tensor.matmul(
                        ps[:, 0:DG],
                        lhsT=xv_c[:, ktl, m * P : (m + 1) * P],
                        rhs=wv_sb[:, ktl, :],
                        start=(ktl == 0),
                        stop=False,
                    )
                nc.tensor.matmul(
                    ps[:, 0:DG],
                    lhsT=ones2[0:1, 0:P],
                    rhs=vb_sb[:],
                    start=False,
                    stop=True,
                )
                nc.vector.tensor_copy(
                    v_sb[:, st, :].rearrange("p (h x) -> p h x", h=HPG)[
                        :, :, 0:HD
                    ],
                    ps[:, 0:DG].rearrange("p (h x) -> p h x", h=HPG),
                )

        # out-projection for one (st, nch) quarter; 4 accumulating matmuls
        osb2_live = {}

        def emit_outproj_part(st, nch):
            if st not in osb2_live:
                osb2_live[st] = outpool.tile(
                    [P, D], bf16, tag="outsb", name=f"outsb_{st}"
                )
            osb2 = osb2_live[st]
            op_ps = scr_pool.tile([P, 512], f32, tag="ps", name=f"op_{st}_{nch}")
            for h_ in range(HPG):
                nc.tensor.matmul(
                    op_ps[:],
                    lhsT=otn_sb[:, h_, st * P : (st + 1) * P],
                    rhs=wo_sb[:, h_, nch * 512 : (nch + 1) * 512],
                    start=(h_ == 0),
                    stop=(h_ == HPG - 1),
                )
            nc.vector.tensor_copy(osb2[:, nch * 512 : (nch + 1) * 512], op_ps[:])
            if nch == 1:
                nc.sync.dma_start(out.ap()[st * P : (st + 1) * P, :], osb2[:])
                del osb2_live[st]

        # ---- attention ---------------------------------------------------
        def make_head(qc, h):
            """Returns (sc, pv, norm) emitters for one (qc, h)."""
            t, po = h // 2, (h % 2) * HD
            o_ps = ops_pool.tile([HD + 1, QW], f32, tag="ops", name=f"o_{qc}_{h}")
            pts = {}

            def sc(kt):
                s_ps = sps_pool.tile(
                    [P, QW], f32, tag="sps", name=f"s_{qc}_{h}_{kt}"
                )
                for hf in range(2):
                    nc.tensor.matmul(
                        s_ps[:, hf * 512 : (hf + 1) * 512],
                        lhsT=kT_sb[po : po + HD, t, kt * P : (kt + 1) * P],
                        rhs=qT_sb[
                            po : po + HD,
                            t,
                            qc * QW + hf * 512 : qc * QW + (hf + 1) * 512,
                        ],
                        start=True,
                        stop=True,
                    )
                pt = ptpool.tile([P, QW], bf16, tag="p", name=f"p_{qc}_{h}_{kt}")
                nc.scalar.activation(pt[:], s_ps[:], EXP, scale=0.125)
                nc.vector.tensor_mul(
                    pt[:], pt[:], mask_sb[:, kt, qc * QW : (qc + 1) * QW]
                )
                pts[kt] = pt

            def pv(kt):
                pt = pts.pop(kt)
                for hf in range(2):
                    nc.tensor.matmul(
                        o_ps[:, hf * 512 : (hf + 1) * 512],
                        lhsT=v_sb[:, kt, h * 65 : (h + 1) * 65],
                        rhs=pt[:, hf * 512 : (hf + 1) * 512],
                        start=(kt == 0),
                        stop=(kt == KT - 1),
                    )

            def norm():
                # baseline normalize: approx-recip of the denominator row,
                # DRAM-bounce broadcast, one TT multiply.
                rec65 = npool.tile([HD + 1, QW], f32, tag="rec")
                nc.vector.reciprocal_approx_fast(out=rec65[:], in_=o_ps[:])
                osb = npool.tile([HD, QW], f32, tag="osb")
                nc.vector.tensor_copy(osb[:], o_ps[0:HD, :])
                scr = dpool.tile([1, QW], f32, tag="scr", name=f"sc_{qc}_{h}")
                nc.sync.dma_start(scr[:], rec65[HD : HD + 1, :])
                rb = npool.tile([HD, QW], f32, tag="rb")
                nc.sync.dma_start(rb[:], scr[:].to_broadcast((HD, QW)))
                nc.vector.tensor_mul(
                    otn_sb[:, h, qc * QW : (qc + 1) * QW], osb[:], rb[:]
                )

            return sc, pv, norm

        def emit_head(qc, h, fillers=()):
            """Full head with lag-2 sc->pv pipeline + optional PE fillers."""
            sc, pv, norm = make_head(qc, h)
            fill = list(fillers)
            for kt in range(KT):
                sc(kt)
                if kt >= 2:
                    pv(kt - 2)
                if kt % 4 == 1 and fill:
                    st, nch = fill.pop(0)
                    emit_outproj_part(st, nch)
            pv(KT - 2)
            pv(KT - 1)
            norm()

        # ---- main emission ----------------------------------------------
        x0 = emit_x_dma(0)
        emit_mask_dma(range(0, 8), 0)
        emit_proj_chunk(0, *x0)
        x1 = emit_x_dma(1)
        emit_mask_dma(range(8, KT), 0)

        # head 0 of qc 0, split around chunk-1 projections
        sc0, pv0, norm0 = make_head(0, 0)
        for kt in range(8):
            sc0(kt)
            if kt >= 2:
                pv0(kt - 2)
        emit_proj_chunk(1, *x1)
        for kt in range(8, KT):
            sc0(kt)
            pv0(kt - 2)
        pv0(KT - 2)
        pv0(KT - 1)
        norm0()

        for h in range(1, HPG):
            emit_head(0, h)
        emit_mask_dma(range(KT), 1)

        # qc 1 with out-proj(qc0: st 0..7) interleaved as PE filler
        fill = [(st, nch) for st in range(S // P // 2) for nch in range(2)]
        for h in range(HPG):
            emit_head(1, h, fillers=fill[h * 4 : (h + 1) * 4])
        # remaining qc0 fillers (if any) then qc1 out-proj tail
        for st, nch in fill[HPG * 4 :]:
            emit_outproj_part(st, nch)
        for st in range(S // P // 2, S // P):
            emit_outproj_part(st, 0)
            emit_outproj_part(st, 1)

    nc.compile()
    return nc


@functools.lru_cache(maxsize=1)
def _graph():
    return build_graph()


def make_in_maps(
    query, key, value, mask,
    wq_kernel, wq_bias, wk_kernel, wk_bias,
    wv_kernel, wv_bias, wo_kernel, wo_bias,
):
    q = np.asarray(query, np.float32)
    k = np.asarray(key, np.float32)
    v = np.asarray(value, np.float32)
    mask = np.asarray(mask)
    wqk = np.asarray(wq_kernel, np.float32)
    wkk = np.asarray(wk_kernel, np.float32)
    wvk = np.asarray(wv_kernel, np.float32)
    wok = np.asarray(wo_kernel, np.float32)

    def tile_x(a):  # [S, D] -> [P, TDIN, S] pre-tiled transpose
        return np.ascontiguousarray(
            a.T.reshape(TDIN, P, S).transpose(1, 0, 2)
        ).astype(BF16)

    xt = [[tile_x(x[b]) for x in (q, k, v)] for b in range(B)]
    mt = [
        np.ascontiguousarray(mask[b].T.astype(np.float32)).astype(BF16)
        for b in range(B)
    ]
    in_maps = []
    for c in range(NCORES):
        b, g = divmod(c, GH)
        cs = slice(g * DG, (g + 1) * DG)
        wo_arr = np.ascontiguousarray(
            wok[cs, :].reshape(HPG, HD, D).transpose(1, 0, 2)
        ).astype(BF16)
        in_maps.append(
            {
                "xq_t": xt[b][0],
                "xk_t": xt[b][1],
                "xv_t": xt[b][2],
                "mask_t": mt[b],
                "wq": np.ascontiguousarray(wqk[:, cs].reshape(TDIN, P, DG).transpose(1, 0, 2)).astype(BF16),
                "wk": np.ascontiguousarray(wkk[:, cs].reshape(TDIN, P, DG).transpose(1, 0, 2)).astype(BF16),
                "wv": np.ascontiguousarray(wvk[:, cs].reshape(TDIN, P, DG).transpose(1, 0, 2)).astype(BF16),
                "wo": wo_arr,
                "qb": np.asarray(wq_bias, np.float32)[cs].reshape(1, DG).astype(BF16),
                "kb": np.asarray(wk_bias, np.float32)[cs].reshape(1, DG).astype(BF16),
                "vb": np.asarray(wv_bias, np.float32)[cs].reshape(1, DG).astype(BF16),
            }
        )
    return in_maps


def combine_outputs(results, wo_bias):
    outs = np.stack([np.asarray(r["out"], np.float32) for r in results])
    full = outs.reshape(B, GH, S, D).sum(axis=1)
    return (full + np.asarray(wo_bias, np.float32)[None, None, :]).astype(
        np.float32
    )


def kernel(**inputs):
    from concourse import bass_utils

    nc = _graph()
    in_maps = make_in_maps(**inputs)
    res = bass_utils.run_bass_kernel_spmd(
        nc, in_maps, core_ids=list(range(NCORES))
    )
    return combine_outputs(res.results, inputs["wo_bias"])
